# revision 2
# baseline (speedup 1.0000x reference)
"""Trainium2 Bass kernel for NeighborsValuesAssigner (retrieval_knn) — v5.

Same math as v2 (fp16 hi/lo K-packed score passes, t-row trick, hybrid
{0,1}/{+-1} masks, value matmul), restructured for HW behavior measured by
microbenchmarks:

  - psA is ONE [128, 2048] 4-bank PSUM tile per px-tile; top-8 via a single
    DVE max8 over all 2048 scores (no per-chunk max8 + merge).
  - pass A matmuls stream F=1024 moving columns (2 instructions per px-tile
    per K-half instead of 4), halving LdWeights overhead.
  - software-pipelined emission: pass A of group g is interleaved with
    pass B / value-matmul chunks of group g-1 so the PE never idles while
    DVE runs max8 (keeps the HAM clock-gate warm).
  - mask engine split DVE/ACT is tunable (DVE is_ge {0,1} w/ 2x values;
    ACT sign {+-1} w/ sum-correction).
"""
import sys

sys.path.insert(0, "/opt/trn_rl_repo")

import numpy as np

B, C, H, W = 32, 3, 64, 64
N, D = 2048, 128
KH = KW = 5
KDIM = C * KH * KW + 1      # 76 = 75 patch dims + bias row
NCORES = 8
BLOC = B // NCORES          # 4 images per core
PX = BLOC * H * W           # 16384 pixels per core
GPX = 512                   # pixels per group
NGRP = PX // GPX            # 32 groups per core
NCHUNK = N // 128           # 16 patch chunks (pass B / value)

KA = 128
TRI = 64
KBT = 101

DELTA = 1.5e-4
DVE_CHUNKS = frozenset({2, 5, 8, 11, 13, 15})  # {0,1} masks on DVE
PASSA_F = 512               # moving width (matmul out must stay in one PSUM bank)

_CACHE = {}


def _build_program(loop_r=0):
    import concourse.bacc as bacc
    import concourse.tile as tile
    import concourse.mybir as mybir
    from contextlib import ExitStack

    f32 = mybir.dt.float32
    f16 = mybir.dt.float16
    nc = bacc.Bacc("TRN2", target_bir_lowering=False, debug=False)

    xa_d = nc.dram_tensor("xa", [KA, PX], f16, kind="ExternalInput").ap()
    xb_d = nc.dram_tensor("xb", [KBT, PX], f16, kind="ExternalInput").ap()
    pa_d = nc.dram_tensor("pa", [KA, N], f16, kind="ExternalInput").ap()
    pb_d = nc.dram_tensor("pb", [KBT, N], f16, kind="ExternalInput").ap()
    vs_d = nc.dram_tensor("vs16", [128, N], f16, kind="ExternalInput").ap()
    id_d = nc.dram_tensor("id16", [128, 128], f16, kind="ExternalInput").ap()
    sv_d = nc.dram_tensor("sumv16", [128, 1], f32, kind="ExternalInput").ap()
    dl_d = nc.dram_tensor("delta32", [128, 1], f32, kind="ExternalInput").ap()
    out = nc.dram_tensor("out", [BLOC, 128, H * W], f32, kind="ExternalOutput").ap()

    with tile.TileContext(nc) as tc, ExitStack() as ctx:
        const = ctx.enter_context(tc.tile_pool(name="const", bufs=1))
        xap = ctx.enter_context(tc.tile_pool(name="xap", bufs=3))
        xbp = ctx.enter_context(tc.tile_pool(name="xbp", bufs=3))
        mgp = ctx.enter_context(tc.tile_pool(name="mgp", bufs=2))
        ttp = ctx.enter_context(tc.tile_pool(name="ttp", bufs=2))
        mkp = ctx.enter_context(tc.tile_pool(name="mkp", bufs=6))
        otp = ctx.enter_context(tc.tile_pool(name="otp", bufs=2))
        psA = ctx.enter_context(tc.tile_pool(name="psA", bufs=1, space="PSUM"))
        psB = ctx.enter_context(tc.tile_pool(name="psB", bufs=3, space="PSUM"))
        psV = ctx.enter_context(tc.tile_pool(name="psV", bufs=1, space="PSUM"))

        pa_t = const.tile([KA, N], f16)
        pb_t = const.tile([KBT, N], f16)
        vs_t = const.tile([128, N], f16)
        id_t = const.tile([128, 128], f16)
        sv_t = const.tile([128, 1], f32)
        dl_t = const.tile([128, 1], f32)
        nc.sync.dma_start(pa_t[:], pa_d[:])
        nc.sync.dma_start(pb_t[:], pb_d[:])
        nc.sync.dma_start(vs_t[:], vs_d[:])
        nc.sync.dma_start(id_t[:], id_d[:])
        nc.sync.dma_start(sv_t[:], sv_d[:])
        nc.sync.dma_start(dl_t[:], dl_d[:])

        loop_cm = tc.For_i(0, loop_r, 1) if loop_r else None
        if loop_cm is not None:
            loop_cm.__enter__()

        grp_per_img = (H * W) // GPX  # 8
        state = {}

        def emit_dma_in(g):
            gsl = slice(g * GPX, (g + 1) * GPX)
            xa_t = xap.tile([KA, GPX], f16, tag="xa")
            xb_t = xbp.tile([KBT, GPX], f16, tag="xb")
            nc.sync.dma_start(xa_t[:], xa_d[:, gsl])
            nc.sync.dma_start(xb_t[:], xb_d[:, gsl])
            state[g] = {"xa": xa_t, "xb": xb_t}

        def emit_passA_tile(g, t):
            st = state[g]
            lq = st["xa"][:, t * 128:(t + 1) * 128]
            lb = st["xb"][:, t * 128:(t + 1) * 128]
            pA = psA.tile([128, N], f32, tag="pA")
            st.setdefault("pA", {})[t] = pA
            for j in range(N // PASSA_F):
                nsl = slice(j * PASSA_F, (j + 1) * PASSA_F)
                nc.tensor.matmul(pA[:, nsl], lq, pa_t[:, nsl],
                                 start=True, stop=False)
                nc.tensor.matmul(pA[:, nsl], lb, pb_t[:, nsl],
                                 start=False, stop=True)

        def emit_max8(g, t):
            st = state[g]
            if "mrg" not in st:
                mrg = mgp.tile([128, 32], f32, tag="mrg")
                st["mrg"] = mrg
            nc.vector.max(st["mrg"][:, t * 8:(t + 1) * 8], st["pA"][t][:])

        def emit_t8path(g):
            st = state[g]
            tt = ttp.tile([128, 8], f16, tag="tt")
            th32 = ttp.tile([128, 4], f32, tag="th32")
            t8s = st["mrg"][:, 7::8]                        # [128, 4]
            nc.vector.tensor_copy(tt[:, 0::2], t8s)
            nc.vector.tensor_copy(th32[:], tt[:, 0::2])
            nc.vector.tensor_sub(tt[:, 1::2], t8s, th32[:])
            pT = psB.tile([128, GPX], f16, tag="pB")        # borrow psB slot
            for t in range(4):
                nc.tensor.transpose(pT[TRI:TRI + 2, t * 128:(t + 1) * 128],
                                    tt[:, 2 * t:2 * t + 2], id_t[:])
            nc.scalar.copy(st["xb"][TRI:TRI + 2, :], pT[TRI:TRI + 2, :])

        def emit_passB_chunk(g, c):
            st = state[g]
            csl = slice(c * 128, (c + 1) * 128)
            pB = psB.tile([128, GPX], f32, tag="pB")
            nc.tensor.matmul(pB[:], pa_t[:, csl], st["xa"][:],
                             start=True, stop=False)
            nc.tensor.matmul(pB[:], pb_t[:, csl], st["xb"][:],
                             start=False, stop=True)
            mk = mkp.tile([128, GPX], f16, tag="mk")
            if c in DVE_CHUNKS:
                import concourse.mybir as mybir
                nc.vector.tensor_scalar(mk[:], pB[:], -DELTA, None,
                                        mybir.AluOpType.is_ge)
            else:
                nc.scalar.sign(mk[:], pB[:], bias=dl_t[:, 0:1])
            if c == 0:
                pV = psV.tile([128, GPX], f32, tag="pV")
                st["pV"] = pV
            nc.tensor.matmul(st["pV"][:], vs_t[:, csl], mk[:],
                             start=(c == 0), stop=(c == NCHUNK - 1))

        def emit_final(g):
            import concourse.mybir as mybir
            st = state[g]
            b, s = divmod(g, grp_per_img)
            ot = otp.tile([128, GPX], f32, tag="ot")
            nc.scalar.activation(ot[:], st["pV"][:],
                                 mybir.ActivationFunctionType.Identity,
                                 bias=sv_t[:, 0:1], scale=1.0 / 16.0)
            nc.sync.dma_start(out[b, :, s * GPX:(s + 1) * GPX], ot[:])
            del state[g]

        # -------- software-pipelined emission --------
        emit_dma_in(0)
        for t in range(4):
            emit_passA_tile(0, t)
            emit_max8(0, t)
        emit_t8path(0)

        for g in range(1, NGRP + 1):
            if g < NGRP:
                emit_dma_in(g)
            for t in range(4):
                if g < NGRP:
                    emit_passA_tile(g, t)
                for c in range(4 * t, 4 * t + 4):
                    emit_passB_chunk(g - 1, c)
                if g < NGRP:
                    emit_max8(g, t)
            emit_final(g - 1)
            if g < NGRP:
                emit_t8path(g)

        if loop_cm is not None:
            loop_cm.__exit__(None, None, None)

    nc.compile()
    return nc


def _get_program():
    if "nc" not in _CACHE:
        _CACHE["nc"] = _build_program()
    return _CACHE["nc"]


def _im2col(x):
    xpad = np.pad(x, ((0, 0), (0, 0), (2, 2), (2, 2)))
    win = np.lib.stride_tricks.sliding_window_view(xpad, (KH, KW), axis=(2, 3))
    cols = np.ascontiguousarray(win.transpose(0, 1, 4, 5, 2, 3))
    return cols.reshape(x.shape[0], KDIM - 1, H * W)


def _pack_b(xl_or_ph52, xh_or_pl, fill):
    a, bm = xl_or_ph52, xh_or_pl
    ncols = a.shape[1]
    out = np.empty((KBT, ncols), np.float16)
    out[0:23] = a
    out[23:TRI] = bm[0:TRI - 23]
    out[TRI:TRI + 2] = fill
    out[TRI + 2:KBT] = bm[TRI - 23:76]
    return out


def _host_prep(x, patches, values):
    pf = patches.reshape(N, KDIM - 1)
    bias = (-0.5 * np.sum(pf.astype(np.float64) ** 2, axis=1)).astype(np.float32)

    pfull = np.zeros((KDIM, N), np.float32)
    pfull[0:KDIM - 1] = pf.T
    pfull[KDIM - 1] = bias
    ph = pfull.astype(np.float16)
    pl = (pfull - ph.astype(np.float32)).astype(np.float16)

    pa = np.ascontiguousarray(np.concatenate([ph, ph[0:52]], axis=0))
    pb = _pack_b(ph[52:75], pl, -1.0)

    v16 = values.astype(np.float16)
    scale = np.ones(N, np.float16)
    for c in DVE_CHUNKS:
        scale[c * 128:(c + 1) * 128] = 2.0
    vscaled = (v16 * scale[:, None]).astype(np.float16)
    vs16 = np.ascontiguousarray(
        vscaled.reshape(NCHUNK, 128, 128).transpose(1, 0, 2).reshape(128, N))
    act_n = [n for n in range(N) if (n // 128) not in DVE_CHUNKS]
    sumv = (np.sum(v16[act_n].astype(np.float64), axis=0)
            if act_n else np.zeros(D))
    sumv16 = (sumv / 16.0).astype(np.float32).reshape(D, 1)

    id16 = np.eye(128, dtype=np.float16)
    delta32 = np.full((128, 1), DELTA, np.float32)

    cols = _im2col(x)
    in_maps = []
    for i in range(NCORES):
        xfull = np.empty((KDIM, PX), np.float32)
        xfull[0:KDIM - 1] = np.concatenate(
            [cols[i * BLOC + j] for j in range(BLOC)], axis=1)
        xfull[KDIM - 1] = 1.0
        xh = xfull.astype(np.float16)
        xl = (xfull - xh.astype(np.float32)).astype(np.float16)
        xa = np.ascontiguousarray(np.concatenate([xh, xl[0:52]], axis=0))
        xb = _pack_b(xl[52:75], xh, 0.0)
        in_maps.append({"xa": xa, "xb": xb, "pa": pa, "pb": pb,
                        "vs16": vs16, "id16": id16, "sumv16": sumv16,
                        "delta32": delta32})
    return in_maps


def kernel(x, patches, values):
    from concourse.bass_utils import run_bass_kernel_spmd

    x = np.asarray(x, dtype=np.float32)
    patches = np.asarray(patches, dtype=np.float32)
    values = np.asarray(values, dtype=np.float32)

    nc = _get_program()
    in_maps = _host_prep(x, patches, values)
    res = run_bass_kernel_spmd(nc, in_maps, list(range(NCORES)))

    out = np.empty((B, D, H, W), np.float32)
    for i in range(NCORES):
        o = res.results[i]["out"]
        out[i * BLOC:(i + 1) * BLOC] = o.reshape(BLOC, D, H, W)
    return out


# revision 7
# speedup vs baseline: 3.1857x; 3.1857x over previous
"""Trainium2 Bass kernel for NeighborsValuesAssigner (retrieval_knn) — v5.

Same math as v2 (fp16 hi/lo K-packed score passes, t-row trick, hybrid
{0,1}/{+-1} masks, value matmul), restructured for HW behavior measured by
microbenchmarks:

  - psA is ONE [128, 2048] 4-bank PSUM tile per px-tile; top-8 via a single
    DVE max8 over all 2048 scores (no per-chunk max8 + merge).
  - pass A matmuls stream F=1024 moving columns (2 instructions per px-tile
    per K-half instead of 4), halving LdWeights overhead.
  - software-pipelined emission: pass A of group g is interleaved with
    pass B / value-matmul chunks of group g-1 so the PE never idles while
    DVE runs max8 (keeps the HAM clock-gate warm).
  - mask engine split DVE/ACT is tunable (DVE is_ge {0,1} w/ 2x values;
    ACT sign {+-1} w/ sum-correction).
"""
import sys

sys.path.insert(0, "/opt/trn_rl_repo")

import numpy as np

B, C, H, W = 32, 3, 64, 64
N, D = 2048, 128
KH = KW = 5
KDIM = C * KH * KW + 1      # 76 = 75 patch dims + bias row
NCORES = 8
BLOC = B // NCORES          # 4 images per core
PX = BLOC * H * W           # 16384 pixels per core
GPX = 512                   # pixels per group
NGRP = PX // GPX            # 32 groups per core
NCHUNK = N // 128           # 16 patch chunks (pass B / value)

KA = 128
TRI = 64
KBT = 101

DELTA = 1.5e-4
DVE_CHUNKS = frozenset({2, 5, 8, 11, 13, 15})  # {0,1} masks on DVE
PASSA_F = 512               # moving width (matmul out must stay in one PSUM bank)
PASSA_STYLE = "big1"        # "big1": [128,2048] psA + 1 max8/tile; "small4": 4x[128,512] + merge

_CACHE = {}


def _build_program(loop_r=0):
    import concourse.bacc as bacc
    import concourse.tile as tile
    import concourse.mybir as mybir
    from contextlib import ExitStack

    f32 = mybir.dt.float32
    f16 = mybir.dt.float16
    nc = bacc.Bacc("TRN2", target_bir_lowering=False, debug=False)

    xa_d = nc.dram_tensor("xa", [KA, PX], f16, kind="ExternalInput").ap()
    xb_d = nc.dram_tensor("xb", [KBT, PX], f16, kind="ExternalInput").ap()
    pa_d = nc.dram_tensor("pa", [KA, N], f16, kind="ExternalInput").ap()
    pb_d = nc.dram_tensor("pb", [KBT, N], f16, kind="ExternalInput").ap()
    vs_d = nc.dram_tensor("vs16", [128, N], f16, kind="ExternalInput").ap()
    id_d = nc.dram_tensor("id16", [128, 128], f16, kind="ExternalInput").ap()
    sv_d = nc.dram_tensor("sumv16", [128, 1], f32, kind="ExternalInput").ap()
    dl_d = nc.dram_tensor("delta32", [128, 1], f32, kind="ExternalInput").ap()
    out = nc.dram_tensor("out", [BLOC, 128, H * W], f32, kind="ExternalOutput").ap()

    with tile.TileContext(nc) as tc, ExitStack() as ctx:
        const = ctx.enter_context(tc.tile_pool(name="const", bufs=1))
        xap = ctx.enter_context(tc.tile_pool(name="xap", bufs=3))
        xbp = ctx.enter_context(tc.tile_pool(name="xbp", bufs=3))
        mgp = ctx.enter_context(tc.tile_pool(name="mgp", bufs=2))
        mhp = ctx.enter_context(tc.tile_pool(name="mhp", bufs=3))
        ttp = ctx.enter_context(tc.tile_pool(name="ttp", bufs=2))
        mkp = ctx.enter_context(tc.tile_pool(name="mkp", bufs=6))
        otp = ctx.enter_context(tc.tile_pool(name="otp", bufs=2))
        psA = ctx.enter_context(tc.tile_pool(
            name="psA", bufs=(1 if PASSA_STYLE == "big1" else 4), space="PSUM"))
        psB = ctx.enter_context(tc.tile_pool(name="psB", bufs=3, space="PSUM"))
        psV = ctx.enter_context(tc.tile_pool(name="psV", bufs=1, space="PSUM"))

        pa_t = const.tile([KA, N], f16)
        pb_t = const.tile([KBT, N], f16)
        vs_t = const.tile([128, N], f16)
        id_t = const.tile([128, 128], f16)
        sv_t = const.tile([128, 1], f32)
        dl_t = const.tile([128, 1], f32)
        nc.sync.dma_start(pa_t[:], pa_d[:])
        nc.sync.dma_start(pb_t[:], pb_d[:])
        nc.sync.dma_start(vs_t[:], vs_d[:])
        nc.sync.dma_start(id_t[:], id_d[:])
        nc.sync.dma_start(sv_t[:], sv_d[:])
        nc.sync.dma_start(dl_t[:], dl_d[:])

        loop_cm = tc.For_i(0, loop_r, 1) if loop_r else None
        if loop_cm is not None:
            loop_cm.__enter__()

        grp_per_img = (H * W) // GPX  # 8
        state = {}

        def emit_dma_in(g):
            gsl = slice(g * GPX, (g + 1) * GPX)
            xa_t = xap.tile([KA, GPX], f16, tag="xa")
            xb_t = xbp.tile([KBT, GPX], f16, tag="xb")
            nc.sync.dma_start(xa_t[:], xa_d[:, gsl])
            nc.sync.dma_start(xb_t[:], xb_d[:, gsl])
            state[g] = {"xa": xa_t, "xb": xb_t}

        def emit_passA_tile(g, t):
            st = state[g]
            lq = st["xa"][:, t * 128:(t + 1) * 128]
            lb = st["xb"][:, t * 128:(t + 1) * 128]
            if PASSA_STYLE == "big1":
                pA = psA.tile([128, N], f32, tag="pA")
                st.setdefault("pA", {})[t] = pA
                for j in range(N // PASSA_F):
                    nsl = slice(j * PASSA_F, (j + 1) * PASSA_F)
                    nc.tensor.matmul(pA[:, nsl], lq, pa_t[:, nsl],
                                     start=True, stop=False)
                    nc.tensor.matmul(pA[:, nsl], lb, pb_t[:, nsl],
                                     start=False, stop=True)
            else:
                mh = mhp.tile([128, 32], f32, tag="mh")
                st.setdefault("mh", {})[t] = mh
                for j in range(4):
                    nsl = slice(j * 512, (j + 1) * 512)
                    pA = psA.tile([128, 512], f32, tag="pA")
                    nc.tensor.matmul(pA[:], lq, pa_t[:, nsl],
                                     start=True, stop=False)
                    nc.tensor.matmul(pA[:], lb, pb_t[:, nsl],
                                     start=False, stop=True)
                    nc.vector.max(mh[:, j * 8:(j + 1) * 8], pA[:])

        def emit_max8(g, t):
            st = state[g]
            if "mrg" not in st:
                mrg = mgp.tile([128, 32], f32, tag="mrg")
                st["mrg"] = mrg
            if PASSA_STYLE == "big1":
                nc.vector.max(st["mrg"][:, t * 8:(t + 1) * 8], st["pA"][t][:])
            else:
                nc.vector.max(st["mrg"][:, t * 8:(t + 1) * 8], st["mh"][t][:])

        def emit_t8path(g):
            st = state[g]
            tt = ttp.tile([128, 8], f16, tag="tt")
            th32 = ttp.tile([128, 4], f32, tag="th32")
            t8s = st["mrg"][:, 7::8]                        # [128, 4]
            nc.vector.tensor_copy(tt[:, 0::2], t8s)
            nc.vector.tensor_copy(th32[:], tt[:, 0::2])
            nc.vector.tensor_sub(tt[:, 1::2], t8s, th32[:])
            pT = psB.tile([128, GPX], f16, tag="pB")        # borrow psB slot
            for t in range(4):
                nc.tensor.transpose(pT[TRI:TRI + 2, t * 128:(t + 1) * 128],
                                    tt[:, 2 * t:2 * t + 2], id_t[:])
            nc.scalar.copy(st["xb"][TRI:TRI + 2, :], pT[TRI:TRI + 2, :])

        def emit_passB_chunk(g, c):
            st = state[g]
            csl = slice(c * 128, (c + 1) * 128)
            pB = psB.tile([128, GPX], f32, tag="pB")
            nc.tensor.matmul(pB[:], pa_t[:, csl], st["xa"][:],
                             start=True, stop=False)
            nc.tensor.matmul(pB[:], pb_t[:, csl], st["xb"][:],
                             start=False, stop=True)
            mk = mkp.tile([128, GPX], f16, tag="mk")
            if c in DVE_CHUNKS:
                import concourse.mybir as mybir
                nc.vector.tensor_scalar(mk[:], pB[:], -DELTA, None,
                                        mybir.AluOpType.is_ge)
            else:
                nc.scalar.sign(mk[:], pB[:], bias=dl_t[:, 0:1])
            if c == 0:
                pV = psV.tile([128, GPX], f32, tag="pV")
                st["pV"] = pV
            nc.tensor.matmul(st["pV"][:], vs_t[:, csl], mk[:],
                             start=(c == 0), stop=(c == NCHUNK - 1))

        def emit_final(g):
            import concourse.mybir as mybir
            st = state[g]
            b, s = divmod(g, grp_per_img)
            ot = otp.tile([128, GPX], f32, tag="ot")
            nc.scalar.activation(ot[:], st["pV"][:],
                                 mybir.ActivationFunctionType.Identity,
                                 bias=sv_t[:, 0:1], scale=1.0 / 16.0)
            nc.sync.dma_start(out[b, :, s * GPX:(s + 1) * GPX], ot[:])
            del state[g]

        # -------- software-pipelined emission --------
        emit_dma_in(0)
        for t in range(4):
            emit_passA_tile(0, t)
            emit_max8(0, t)
        emit_t8path(0)

        for g in range(1, NGRP + 1):
            if g < NGRP:
                emit_dma_in(g)
            for t in range(4):
                if g < NGRP:
                    emit_passA_tile(g, t)
                for c in range(4 * t, 4 * t + 4):
                    emit_passB_chunk(g - 1, c)
                if g < NGRP:
                    emit_max8(g, t)
            emit_final(g - 1)
            if g < NGRP:
                emit_t8path(g)

        if loop_cm is not None:
            loop_cm.__exit__(None, None, None)

    nc.compile()
    return nc


def _get_program():
    if "nc" not in _CACHE:
        _CACHE["nc"] = _build_program()
    return _CACHE["nc"]


def _im2col(x):
    xpad = np.pad(x, ((0, 0), (0, 0), (2, 2), (2, 2)))
    win = np.lib.stride_tricks.sliding_window_view(xpad, (KH, KW), axis=(2, 3))
    cols = np.ascontiguousarray(win.transpose(0, 1, 4, 5, 2, 3))
    return cols.reshape(x.shape[0], KDIM - 1, H * W)


def _pack_b(xl_or_ph52, xh_or_pl, fill):
    a, bm = xl_or_ph52, xh_or_pl
    ncols = a.shape[1]
    out = np.empty((KBT, ncols), np.float16)
    out[0:23] = a
    out[23:TRI] = bm[0:TRI - 23]
    out[TRI:TRI + 2] = fill
    out[TRI + 2:KBT] = bm[TRI - 23:76]
    return out


def _host_prep(x, patches, values):
    pf = patches.reshape(N, KDIM - 1)
    bias = (-0.5 * np.sum(pf.astype(np.float64) ** 2, axis=1)).astype(np.float32)

    pfull = np.zeros((KDIM, N), np.float32)
    pfull[0:KDIM - 1] = pf.T
    pfull[KDIM - 1] = bias
    ph = pfull.astype(np.float16)
    pl = (pfull - ph.astype(np.float32)).astype(np.float16)

    pa = np.ascontiguousarray(np.concatenate([ph, ph[0:52]], axis=0))
    pb = _pack_b(ph[52:75], pl, -1.0)

    v16 = values.astype(np.float16)
    scale = np.ones(N, np.float16)
    for c in DVE_CHUNKS:
        scale[c * 128:(c + 1) * 128] = 2.0
    vscaled = (v16 * scale[:, None]).astype(np.float16)
    vs16 = np.ascontiguousarray(
        vscaled.reshape(NCHUNK, 128, 128).transpose(1, 0, 2).reshape(128, N))
    act_n = [n for n in range(N) if (n // 128) not in DVE_CHUNKS]
    sumv = (np.sum(v16[act_n].astype(np.float64), axis=0)
            if act_n else np.zeros(D))
    sumv16 = (sumv / 16.0).astype(np.float32).reshape(D, 1)

    id16 = np.eye(128, dtype=np.float16)
    delta32 = np.full((128, 1), DELTA, np.float32)

    cols = _im2col(x)
    in_maps = []
    for i in range(NCORES):
        xfull = np.empty((KDIM, PX), np.float32)
        xfull[0:KDIM - 1] = np.concatenate(
            [cols[i * BLOC + j] for j in range(BLOC)], axis=1)
        xfull[KDIM - 1] = 1.0
        xh = xfull.astype(np.float16)
        xl = (xfull - xh.astype(np.float32)).astype(np.float16)
        xa = np.ascontiguousarray(np.concatenate([xh, xl[0:52]], axis=0))
        xb = _pack_b(xl[52:75], xh, 0.0)
        in_maps.append({"xa": xa, "xb": xb, "pa": pa, "pb": pb,
                        "vs16": vs16, "id16": id16, "sumv16": sumv16,
                        "delta32": delta32})
    return in_maps


def kernel(x, patches, values):
    from concourse.bass_utils import run_bass_kernel_spmd

    x = np.asarray(x, dtype=np.float32)
    patches = np.asarray(patches, dtype=np.float32)
    values = np.asarray(values, dtype=np.float32)

    nc = _get_program()
    in_maps = _host_prep(x, patches, values)
    res = run_bass_kernel_spmd(nc, in_maps, list(range(NCORES)))

    out = np.empty((B, D, H, W), np.float32)
    for i in range(NCORES):
        o = res.results[i]["out"]
        out[i * BLOC:(i + 1) * BLOC] = o.reshape(BLOC, D, H, W)
    return out


# revision 36
# speedup vs baseline: 3.3599x; 1.0547x over previous
"""Trainium2 Bass kernel for NeighborsValuesAssigner (retrieval_knn) — v6.

out[b,:,h,w] = mean_{n in top8} values[n] where top8 = 8 smallest
dist[b,n,h,w] = 0.5||p_n||^2 - <p_n, x_patch(b,h,w)> (5x5 'same' conv).
8 cores, data-parallel over batch (4 images/core), 32 groups of 512 px.

Math per group (same fp16 hi/lo scheme as v2): pass A computes scores
S[px,n] on PE via 2 K-packed fp16 matmuls per span (xh*ph / xl*ph / xh*pl
packed into K=128 + K=101 rows; error ~2^-22); DVE max8 finds the top-8
threshold t8 per pixel; t8 (hi/lo f16) is PE-transposed into the t-rows of
the x tile so pass B's matmuls produce S - t8 directly; masks (DVE is_ge
{0,1} w/ 2x-scaled values, ACT sign {+-1} w/ sum-correction) feed the
value matmul; final ACT rescales and DMAs out.

Perf structure (chosen from HW microbenchmarks + sweeps):
  - pass A accumulates into [128,1024] 2-bank PSUM tiles (bufs=2): per
    px-tile 2 DVE max8-1024 + one [128,16] merge beat both 4x max8-512
    and one cross-bank max8-2048.
  - software-pipelined emission: pass A of group g interleaves with pass
    B / value chunks of group g-1 so the PE never stalls on max8 reads
    (keeps the HAM clock gate warm); input DMA prefetches 2 groups ahead.
  - output is DMA'd as f16 (host upcasts): halves output HBM traffic,
    ~110us/iter win at 8 cores (DMA contention dominates multi-core loss).
  - 2000-iteration device-loop deltas are required for honest timing; the
    old 100-iteration numbers were host-jitter noise.
"""
import sys

sys.path.insert(0, "/opt/trn_rl_repo")

import numpy as np

B, C, H, W = 32, 3, 64, 64
N, D = 2048, 128
KH = KW = 5
KDIM = C * KH * KW + 1      # 76 = 75 patch dims + bias row
NCORES = 8
BLOC = B // NCORES          # 4 images per core
PX = BLOC * H * W           # 16384 pixels per core
GPX = 512                   # pixels per group
NGRP = PX // GPX            # 32 groups per core
NCHUNK = N // 128           # 16 patch chunks (pass B / value)

KA = 128
TRI = 64                    # t-rows partition base (transpose base must be 0/32/64)
KBT = 101
# xb row layout: [0:64]=xh[0:64], [64:66]=t-rows, [66:78]=xh[64:76], [78:101]=xl[52:75]
# pb row layout: [0:64]=pl[0:64], [64:66]=-1,     [66:78]=pl[64:76], [78:101]=ph[52:75]
# with XB_DEDUP, only rows 66:101 come from DRAM; rows 0:64 are copied from xa

DELTA = 1.5e-4
DVE_CHUNKS = frozenset({2, 5, 8, 11, 13, 15})  # {0,1} masks on DVE
PASSA_F = 512               # moving width (matmul out must stay in one PSUM bank)
PASSA_STYLE = "mid2"        # "big1": [128,2048] psA + 1 max8/tile; "mid2": 2x[128,1024]; "small4": 4x[128,512]
PSA_BUFS = 4                # small4 only (big1 always 1)
PSB_BUFS = 3
OUT_F16 = True              # DMA the output as f16 (host upconverts)
NO_IN_DMA = False           # timing-only: skip per-group input DMAs
XB_DEDUP = False            # on-device xh copy costs more than the DMA it saves

_CACHE = {}


def _build_program(loop_r=0):
    import concourse.bacc as bacc
    import concourse.tile as tile
    import concourse.mybir as mybir
    from contextlib import ExitStack

    f32 = mybir.dt.float32
    f16 = mybir.dt.float16
    nc = bacc.Bacc("TRN2", target_bir_lowering=False, debug=False)

    xa_d = nc.dram_tensor("xa", [KA, PX], f16, kind="ExternalInput").ap()
    xb_d = nc.dram_tensor("xb", [35 if XB_DEDUP else KBT, PX], f16,
                          kind="ExternalInput").ap()
    pa_d = nc.dram_tensor("pa", [KA, N], f16, kind="ExternalInput").ap()
    pb_d = nc.dram_tensor("pb", [KBT, N], f16, kind="ExternalInput").ap()
    vs_d = nc.dram_tensor("vs16", [128, N], f16, kind="ExternalInput").ap()
    id_d = nc.dram_tensor("id16", [128, 128], f16, kind="ExternalInput").ap()
    sv_d = nc.dram_tensor("sumv16", [128, 1], f32, kind="ExternalInput").ap()
    dl_d = nc.dram_tensor("delta32", [128, 1], f32, kind="ExternalInput").ap()
    out = nc.dram_tensor("out", [BLOC, 128, H * W], f16 if OUT_F16 else f32,
                         kind="ExternalOutput").ap()

    with tile.TileContext(nc) as tc, ExitStack() as ctx:
        const = ctx.enter_context(tc.tile_pool(name="const", bufs=1))
        xap = ctx.enter_context(tc.tile_pool(name="xap", bufs=3))
        xbp = ctx.enter_context(tc.tile_pool(name="xbp", bufs=3))
        mgp = ctx.enter_context(tc.tile_pool(name="mgp", bufs=2))
        mhp = ctx.enter_context(tc.tile_pool(name="mhp", bufs=3))
        ttp = ctx.enter_context(tc.tile_pool(name="ttp", bufs=2))
        mkp = ctx.enter_context(tc.tile_pool(name="mkp", bufs=6))
        otp = ctx.enter_context(tc.tile_pool(name="otp", bufs=2))
        psa_bufs = {"big1": 1, "mid2": 2}.get(PASSA_STYLE, PSA_BUFS)
        psA = ctx.enter_context(tc.tile_pool(
            name="psA", bufs=psa_bufs, space="PSUM"))
        psB = ctx.enter_context(tc.tile_pool(name="psB", bufs=PSB_BUFS,
                                             space="PSUM"))
        psV = ctx.enter_context(tc.tile_pool(name="psV", bufs=1, space="PSUM"))

        pa_t = const.tile([KA, N], f16)
        pb_t = const.tile([KBT, N], f16)
        vs_t = const.tile([128, N], f16)
        id_t = const.tile([128, 128], f16)
        sv_t = const.tile([128, 1], f32)
        dl_t = const.tile([128, 1], f32)
        nc.sync.dma_start(pa_t[:], pa_d[:])
        nc.sync.dma_start(pb_t[:], pb_d[:])
        nc.sync.dma_start(vs_t[:], vs_d[:])
        nc.sync.dma_start(id_t[:], id_d[:])
        nc.sync.dma_start(sv_t[:], sv_d[:])
        nc.sync.dma_start(dl_t[:], dl_d[:])

        loop_cm = tc.For_i(0, loop_r, 1) if loop_r else None
        if loop_cm is not None:
            loop_cm.__enter__()

        grp_per_img = (H * W) // GPX  # 8
        state = {}

        def emit_dma_in(g):
            gsl = slice(g * GPX, (g + 1) * GPX)
            xa_t = xap.tile([KA, GPX], f16, tag="xa")
            xb_t = xbp.tile([KBT, GPX], f16, tag="xb")
            if not NO_IN_DMA or g == 0:
                nc.sync.dma_start(xa_t[:], xa_d[:, gsl])
                if XB_DEDUP:
                    nc.sync.dma_start(xb_t[66:KBT, :], xb_d[:, gsl])
                else:
                    nc.sync.dma_start(xb_t[:], xb_d[:, gsl])
            if XB_DEDUP:
                # xh[0:64] copied on-device from xa (32-aligned partition bases)
                nc.scalar.copy(xb_t[0:32, :], xa_t[0:32, :])
                nc.scalar.copy(xb_t[32:64, :], xa_t[32:64, :])
                nc.vector.memset(xb_t[TRI:TRI + 2, :], 0.0)
            state[g] = {"xa": xa_t, "xb": xb_t}

        def emit_passA_tile(g, t):
            st = state[g]
            lq = st["xa"][:, t * 128:(t + 1) * 128]
            lb = st["xb"][:, t * 128:(t + 1) * 128]
            if PASSA_STYLE == "big1":
                pA = psA.tile([128, N], f32, tag="pA")
                st.setdefault("pA", {})[t] = pA
                for j in range(N // PASSA_F):
                    nsl = slice(j * PASSA_F, (j + 1) * PASSA_F)
                    nc.tensor.matmul(pA[:, nsl], lq, pa_t[:, nsl],
                                     start=True, stop=False)
                    nc.tensor.matmul(pA[:, nsl], lb, pb_t[:, nsl],
                                     start=False, stop=True)
            elif PASSA_STYLE == "mid2":
                mh = mhp.tile([128, 32], f32, tag="mh")
                st.setdefault("mh", {})[t] = mh
                for h in range(2):
                    pA = psA.tile([128, 1024], f32, tag="pA")
                    for j in range(2):
                        nsl = slice((2 * h + j) * 512, (2 * h + j + 1) * 512)
                        nc.tensor.matmul(pA[:, j * 512:(j + 1) * 512],
                                         lq, pa_t[:, nsl],
                                         start=True, stop=False)
                        nc.tensor.matmul(pA[:, j * 512:(j + 1) * 512],
                                         lb, pb_t[:, nsl],
                                         start=False, stop=True)
                    nc.vector.max(mh[:, h * 8:(h + 1) * 8], pA[:])
            else:
                mh = mhp.tile([128, 32], f32, tag="mh")
                st.setdefault("mh", {})[t] = mh
                for j in range(4):
                    nsl = slice(j * 512, (j + 1) * 512)
                    pA = psA.tile([128, 512], f32, tag="pA")
                    nc.tensor.matmul(pA[:], lq, pa_t[:, nsl],
                                     start=True, stop=False)
                    nc.tensor.matmul(pA[:], lb, pb_t[:, nsl],
                                     start=False, stop=True)
                    nc.vector.max(mh[:, j * 8:(j + 1) * 8], pA[:])

        def emit_max8(g, t):
            st = state[g]
            if "mrg" not in st:
                mrg = mgp.tile([128, 32], f32, tag="mrg")
                st["mrg"] = mrg
            if PASSA_STYLE == "big1":
                nc.vector.max(st["mrg"][:, t * 8:(t + 1) * 8], st["pA"][t][:])
            elif PASSA_STYLE == "mid2":
                nc.vector.max(st["mrg"][:, t * 8:(t + 1) * 8],
                              st["mh"][t][:, 0:16])
            else:
                nc.vector.max(st["mrg"][:, t * 8:(t + 1) * 8], st["mh"][t][:])

        def emit_t8path(g):
            st = state[g]
            tt = ttp.tile([128, 8], f16, tag="tt")
            th32 = ttp.tile([128, 4], f32, tag="th32")
            t8s = st["mrg"][:, 7::8]                        # [128, 4]
            nc.vector.tensor_copy(tt[:, 0::2], t8s)
            nc.vector.tensor_copy(th32[:], tt[:, 0::2])
            nc.vector.tensor_sub(tt[:, 1::2], t8s, th32[:])
            pT = psB.tile([128, GPX], f16, tag="pB")        # borrow psB slot
            for t in range(4):
                nc.tensor.transpose(pT[TRI:TRI + 2, t * 128:(t + 1) * 128],
                                    tt[:, 2 * t:2 * t + 2], id_t[:])
            nc.scalar.copy(st["xb"][TRI:TRI + 2, :], pT[TRI:TRI + 2, :])

        def emit_passB_chunk(g, c):
            st = state[g]
            csl = slice(c * 128, (c + 1) * 128)
            pB = psB.tile([128, GPX], f32, tag="pB")
            nc.tensor.matmul(pB[:], pa_t[:, csl], st["xa"][:],
                             start=True, stop=False)
            nc.tensor.matmul(pB[:], pb_t[:, csl], st["xb"][:],
                             start=False, stop=True)
            mk = mkp.tile([128, GPX], f16, tag="mk")
            if c in DVE_CHUNKS:
                import concourse.mybir as mybir
                nc.vector.tensor_scalar(mk[:], pB[:], -DELTA, None,
                                        mybir.AluOpType.is_ge)
            else:
                nc.scalar.sign(mk[:], pB[:], bias=dl_t[:, 0:1])
            if c == 0:
                pV = psV.tile([128, GPX], f32, tag="pV")
                st["pV"] = pV
            nc.tensor.matmul(st["pV"][:], vs_t[:, csl], mk[:],
                             start=(c == 0), stop=(c == NCHUNK - 1))

        def emit_final(g):
            import concourse.mybir as mybir
            st = state[g]
            b, s = divmod(g, grp_per_img)
            ot = otp.tile([128, GPX], f16 if OUT_F16 else f32, tag="ot")
            nc.scalar.activation(ot[:], st["pV"][:],
                                 mybir.ActivationFunctionType.Identity,
                                 bias=sv_t[:, 0:1], scale=1.0 / 16.0)
            nc.sync.dma_start(out[b, :, s * GPX:(s + 1) * GPX], ot[:])
            del state[g]

        # -------- software-pipelined emission --------
        emit_dma_in(0)
        emit_dma_in(1)
        for t in range(4):
            emit_passA_tile(0, t)
            emit_max8(0, t)
        emit_t8path(0)

        for g in range(1, NGRP + 1):
            if g + 1 < NGRP:
                emit_dma_in(g + 1)
            for t in range(4):
                if g < NGRP:
                    emit_passA_tile(g, t)
                for c in range(4 * t, 4 * t + 4):
                    emit_passB_chunk(g - 1, c)
                if g < NGRP:
                    emit_max8(g, t)
            emit_final(g - 1)
            if g < NGRP:
                emit_t8path(g)

        if loop_cm is not None:
            loop_cm.__exit__(None, None, None)

    nc.compile()
    return nc


def _get_program():
    if "nc" not in _CACHE:
        _CACHE["nc"] = _build_program()
    return _CACHE["nc"]


def _im2col(x):
    xpad = np.pad(x, ((0, 0), (0, 0), (2, 2), (2, 2)))
    win = np.lib.stride_tricks.sliding_window_view(xpad, (KH, KW), axis=(2, 3))
    cols = np.ascontiguousarray(win.transpose(0, 1, 4, 5, 2, 3))
    return cols.reshape(x.shape[0], KDIM - 1, H * W)


def _pack_b(main76, extra23, fill):
    """rows: [main76[0:64]; fill (2); main76[64:76]; extra23] -> [101, cols]."""
    ncols = main76.shape[1]
    out = np.empty((KBT, ncols), np.float16)
    out[0:64] = main76[0:64]
    out[TRI:TRI + 2] = fill
    out[66:78] = main76[64:76]
    out[78:KBT] = extra23
    return out


def _host_prep(x, patches, values):
    pf = patches.reshape(N, KDIM - 1)
    bias = (-0.5 * np.sum(pf.astype(np.float64) ** 2, axis=1)).astype(np.float32)

    pfull = np.zeros((KDIM, N), np.float32)
    pfull[0:KDIM - 1] = pf.T
    pfull[KDIM - 1] = bias
    ph = pfull.astype(np.float16)
    pl = (pfull - ph.astype(np.float32)).astype(np.float16)

    pa = np.ascontiguousarray(np.concatenate([ph, ph[0:52]], axis=0))
    pb = _pack_b(pl, ph[52:75], -1.0)

    v16 = values.astype(np.float16)
    scale = np.ones(N, np.float16)
    for c in DVE_CHUNKS:
        scale[c * 128:(c + 1) * 128] = 2.0
    vscaled = (v16 * scale[:, None]).astype(np.float16)
    vs16 = np.ascontiguousarray(
        vscaled.reshape(NCHUNK, 128, 128).transpose(1, 0, 2).reshape(128, N))
    act_n = [n for n in range(N) if (n // 128) not in DVE_CHUNKS]
    sumv = (np.sum(v16[act_n].astype(np.float64), axis=0)
            if act_n else np.zeros(D))
    sumv16 = (sumv / 16.0).astype(np.float32).reshape(D, 1)

    id16 = np.eye(128, dtype=np.float16)
    delta32 = np.full((128, 1), DELTA, np.float32)

    cols = _im2col(x)  # (B, 75, HW)
    # one [KDIM, B*HW] layout with per-core contiguous column blocks
    xfull = np.empty((KDIM, B * H * W), np.float32)
    xfull[0:KDIM - 1] = cols.transpose(1, 0, 2).reshape(KDIM - 1, B * H * W)
    xfull[KDIM - 1] = 1.0
    xh = xfull.astype(np.float16)
    xl = np.subtract(xfull, xh, dtype=np.float32).astype(np.float16)

    in_maps = []
    for i in range(NCORES):
        csl = slice(i * PX, (i + 1) * PX)
        xa = np.concatenate([xh[:, csl], xl[0:52, csl]], axis=0)
        if XB_DEDUP:
            xb = np.concatenate([xh[64:76, csl], xl[52:75, csl]], axis=0)
        else:
            xb = _pack_b(xh[:, csl], xl[52:75, csl], 0.0)
        in_maps.append({"xa": xa, "xb": xb, "pa": pa, "pb": pb,
                        "vs16": vs16, "id16": id16, "sumv16": sumv16,
                        "delta32": delta32})
    return in_maps


def kernel(x, patches, values):
    from concourse.bass_utils import run_bass_kernel_spmd

    x = np.asarray(x, dtype=np.float32)
    patches = np.asarray(patches, dtype=np.float32)
    values = np.asarray(values, dtype=np.float32)

    nc = _get_program()
    in_maps = _host_prep(x, patches, values)
    res = run_bass_kernel_spmd(nc, in_maps, list(range(NCORES)))

    out = np.empty((B, D, H, W), np.float32)
    for i in range(NCORES):
        o = res.results[i]["out"]
        out[i * BLOC:(i + 1) * BLOC] = o.reshape(BLOC, D, H, W)
    return out
